# revision 43
# baseline (speedup 1.0000x reference)
"""AttnBlock (GroupNorm + 1x1-conv self-attention + residual) on 8 trn2 cores.

Sharding: pure data parallel -- batch b runs on core b (B=8, n_cores=8).
Each core computes its whole batch: GN -> QKV -> attention -> proj -> residual.

Layout (per core, C=512, N=H*W=4096):
  x resident in SBUF as fp16 [P, CT, 512] x 8 chunks (stats, normalize and
  the residual all read it; x is DMA'd exactly once).
  h, Q, K stored [C, N] (channels on partitions), V stored [N, C]; all fp8e4m3.
  S^T = K^T Q computed [keys m, queries n] in fp8 DoubleRow (K=256/matmul);
  merged score tiles [P, 2, 512] (two key-tiles) so each softmax exp is one
  1024-wide ACT instruction; expS stored as exp(s)/16 in fp8 (cancels in A/D).
  Softmax denominator D via DoubleRow matmuls with a ones lhsT (PE, not DVE);
  division deferred: A_norm = pa * (1/D) fused into the PSUM evacuation
  (DVE), so the projection runs on normalized fp8 A and the residual is a
  single add.
  PV runs in two passes: chunks 0-2 interleaved with the S/exp stream
  (3 PSUM banks), chunk 3 after the stream in the denominator's bank (its
  reciprocal frees it) -- exactly 8 PSUM banks with double-buffered merged
  score tiles.
  The output projection of block nb-1 is emitted after pass 2 of block nb so
  the PE never waits on the reciprocal/broadcast chain.

Engine budget per attention block: ACT 16 merged exps; PE S/PV/D/proj
matmuls (all fp8 DoubleRow); DVE evacuations; Pool broadcasts + normalize.
"""

import sys

sys.path.insert(0, "/opt/trn_rl_repo")

import numpy as np

import concourse.bass as bass
from concourse import bacc
import concourse.tile as tile
from concourse import bass_isa, mybir
from concourse.bass import ds
from concourse.bass_utils import run_bass_kernel_spmd

P = 128
C = 512
CT = C // P          # 4 channel tiles
N = 4096
NCH = N // 512       # 8 column chunks of 512
NT = N // P          # 32 position tiles of 128
NP = NT // 2         # 16 merged key-tile pairs
G = 32               # groups
GS = C // G          # 16 channels per group
EPS = 1e-5
SCALE = float(C) ** -0.5

F16 = mybir.dt.float16
F32 = mybir.dt.float32
F8 = mybir.dt.float8e4
LN16 = 2.772588722239781  # ln(16): expS stored as exp(s)/16 in fp8 (cancels in A/D)

TUNE = dict(
    x16_bufs=8,
    acc_bufs=3,
    s_bufs=2,
    es_bufs=20,
    a8_bufs=6,
    ot_bufs=3,
    hch_bufs=3,
    pv_depth=5,      # S->exp->PV pipeline depth, in merged pairs
    rep=1,           # repeat the whole computation inside the program (timing)
)


def build_nc(with_cb: bool, tune=None, has_qkb: bool = False):
    t_ = dict(TUNE)
    if tune:
        t_.update(tune)
    nc = bacc.Bacc(trn_type="TRN2")

    xc16_d = nc.dram_tensor("xc16", [NCH, P, CT, 512], F16, kind="ExternalInput")
    wq_d = nc.dram_tensor("wqT", [C, C], F16, kind="ExternalInput")
    wk_d = nc.dram_tensor("wkT", [C, C], F16, kind="ExternalInput")
    wv_d = nc.dram_tensor("wvT", [C, C], F16, kind="ExternalInput")
    wp_d = nc.dram_tensor("wpT", [C, C], F16, kind="ExternalInput")
    bqk_d = nc.dram_tensor("bqk", [P, 2, CT], F32, kind="ExternalInput")
    gnw_d = nc.dram_tensor("gnw", [P, CT, 2], F32, kind="ExternalInput")
    gstat_d = nc.dram_tensor("gstat", [P, CT, G], F32, kind="ExternalInput")
    gexp_d = nc.dram_tensor("gexp", [P, CT, P], F32, kind="ExternalInput")
    if with_cb:
        cb_d = nc.dram_tensor("cb", [P, CT], F32, kind="ExternalInput")
    out_d = nc.dram_tensor("outc", [NCH, P, CT, 512], F16, kind="ExternalOutput")
    w3 = {
        "q": wq_d.rearrange("(t p) o -> p t o", p=P),
        "k": wk_d.rearrange("(t p) o -> p t o", p=P),
        "v": wv_d.rearrange("(t p) o -> p t o", p=P),
        "p": wp_d.rearrange("(t p) o -> p t o", p=P),
    }

    from contextlib import ExitStack

    DR = mybir.MatmulPerfMode.DoubleRow

    with tile.TileContext(nc) as tc, ExitStack() as ctx:
        persist = ctx.enter_context(tc.tile_pool(name="persist", bufs=1))
        work = ctx.enter_context(tc.tile_pool(name="work", bufs=2))
        # PSUM: exactly 8 banks.
        #   ps_s:   [P,2,512] x2 = 4 banks (merged score tiles / proj outputs)
        #   ps_acc: [P,512]   x3 = 3 banks (QKV accums; PV pass-1 chunks 0-2)
        #   ps_d:   [P,512]   x1 = 1 bank  (GN stats; softmax D; PV pass-2)
        ps_s = ctx.enter_context(
            tc.tile_pool(name="ps_s", bufs=t_["s_bufs"], space="PSUM")
        )
        ps_acc = ctx.enter_context(
            tc.tile_pool(name="ps_acc", bufs=t_["acc_bufs"], space="PSUM")
        )
        ps_d = ctx.enter_context(tc.tile_pool(name="ps_d", bufs=1, space="PSUM"))

        # x chunks stream first in queue order -- everything gates on stats.
        x16 = []
        for nch in range(NCH):
            xt = work.tile(
                [P, CT, 512], F16, tag="x16", bufs=t_["x16_bufs"], name="x16"
            )
            nc.sync.dma_start(out=xt, in_=xc16_d.ap()[nch])
            x16.append(xt)

        # ---------------- persistent weights / constants ----------------
        wsb = {}
        for kk in ("q", "k", "v", "p"):
            wsb[kk] = work.tile([P, CT, C], F16, tag="w16", bufs=4, name=f"w16{kk}")
            nc.sync.dma_start(out=wsb[kk], in_=w3[kk])
        w8 = {}
        for kk in ("q", "k", "v", "p"):
            # fp8 weight copies, converted on the idle Pool engine at startup;
            # consumed by DoubleRow matmuls as channel-tile pairs.
            w8[kk] = persist.tile([P, CT, C], F8, tag=f"w8{kk}", name=f"w8{kk}")
            nc.gpsimd.tensor_copy(w8[kk], wsb[kk])
        bqk_sb = persist.tile([P, 2, CT], F32, tag="bqk")
        nc.sync.dma_start(out=bqk_sb, in_=bqk_d.ap())
        gnw_sb = persist.tile([P, CT, 2], F32, tag="gnw")
        nc.sync.dma_start(out=gnw_sb, in_=gnw_d.ap())
        gstat_sb = persist.tile([P, CT, G], F32, tag="gstat")
        nc.sync.dma_start(out=gstat_sb, in_=gstat_d.ap())
        gexp_sb = persist.tile([P, CT, P], F32, tag="gexp")
        nc.sync.dma_start(out=gexp_sb, in_=gexp_d.ap())
        if with_cb:
            cb_sb = persist.tile([P, CT], F32, tag="cb")
            nc.sync.dma_start(out=cb_sb, in_=cb_d.ap())
        eps_sb = persist.tile([G, 1], F32, tag="eps")
        nc.vector.memset(eps_sb, EPS)
        ones8 = persist.tile([P, 2, 16], F8, tag="ones8")
        nc.vector.memset(ones8, 1.0)
        mln16 = persist.tile([P, 1], F32, tag="mln16")
        nc.vector.memset(mln16, -LN16)

        Q_sb = persist.tile([P, CT, N], F8, tag="Q")
        K_sb = persist.tile([P, CT, N], F8, tag="K")
        V_sb = persist.tile([P, NT, C], F8, tag="V")

        SD = nc.vector.BN_STATS_DIM
        AD = nc.vector.BN_AGGR_DIM

        for _rep in range(t_["rep"]):
            # ---------------- phase 1: GN statistics ----------------
            stats = persist.tile([P, CT - 1, NCH, SD], F32, tag="stats")
            if _rep > 0:
                x16 = []
                for nch in range(NCH):
                    xt = work.tile(
                        [P, CT, 512], F16, tag="x16", bufs=t_["x16_bufs"],
                        name="x16",
                    )
                    nc.sync.dma_start(out=xt, in_=xc16_d.ap()[nch])
                    x16.append(xt)
            # tiles 0..2 on DVE (bn_stats); tile 3 on the otherwise-idle ACT
            # via accum_out (sum + sum of squares).
            asum = persist.tile([P, NCH, 2], F32, tag="asum")
            for nch in range(NCH):
                for t in range(CT - 1):
                    nc.vector.bn_stats(
                        out=stats[:, t, nch, :], in_=x16[nch][:, t, :]
                    )
                ascr = work.tile([P, 512], F16, tag="ascr", bufs=2, name="ascr")
                nc.scalar.activation(
                    ascr,
                    x16[nch][:, CT - 1, :],
                    mybir.ActivationFunctionType.Copy,
                    accum_out=asum[:, nch, 0:1],
                )
                ascr2 = work.tile([P, 512], F16, tag="ascr", bufs=2, name="ascr2")
                nc.scalar.activation(
                    ascr2,
                    x16[nch][:, CT - 1, :],
                    mybir.ActivationFunctionType.Square,
                    accum_out=asum[:, nch, 1:2],
                )
            mv = persist.tile([P, CT, AD], F32, tag="mv")
            rs = persist.tile([P, CT, 2], F32, tag="rs")
            for t in range(CT - 1):
                nc.vector.bn_aggr(out=mv[:, t, :], in_=stats[:, t, :, :])
                # rs = [mean, E[x^2]] per channel
                nc.vector.tensor_mul(rs[:, t, 1:2], mv[:, t, 0:1], mv[:, t, 0:1])
                nc.vector.tensor_add(rs[:, t, 1:2], rs[:, t, 1:2], mv[:, t, 1:2])
                nc.vector.tensor_copy(rs[:, t, 0:1], mv[:, t, 0:1])
            # tile 3: rs = [sum, sumsq]/N from the ACT accumulators
            ssum = persist.tile([P, 2], F32, tag="ssum")
            nc.vector.reduce_sum(
                out=ssum[:, 0:1], in_=asum[:, :, 0], axis=mybir.AxisListType.X
            )
            nc.vector.reduce_sum(
                out=ssum[:, 1:2], in_=asum[:, :, 1], axis=mybir.AxisListType.X
            )
            nc.vector.tensor_scalar_mul(rs[:, CT - 1, :], ssum, 1.0 / N)

            # group reduce: pstat[g, :] = [mu_g, E2_g]  (1/GS folded into gstat)
            pstat_t = ps_d.tile([P, 512], F32, tag="d", name="pstat")
            pstat = pstat_t[:G, 0:2]
            for t in range(CT):
                nc.tensor.matmul(
                    pstat,
                    lhsT=gstat_sb[:, t, :],
                    rhs=rs[:, t, :],
                    start=(t == 0),
                    stop=(t == CT - 1),
                )
            gmr = persist.tile([P, 2], F32, tag="gmr")  # [mu_g, rstd_g], padded
            nc.vector.memset(gmr, 0.0)
            gst = persist.tile([G, 2], F32, tag="gst")  # pstat evacuated to SBUF
            nc.vector.tensor_copy(gst, pstat)
            gvar = persist.tile([G, 1], F32, tag="gvar")
            nc.vector.tensor_copy(gmr[:G, 0:1], gst[:, 0:1])
            nc.vector.tensor_mul(gvar, gst[:, 0:1], gst[:, 0:1])
            nc.vector.tensor_tensor(
                gvar, gst[:, 1:2], gvar, op=mybir.AluOpType.subtract
            )
            nc.scalar.activation(
                gmr[:G, 1:2], gvar, mybir.ActivationFunctionType.Sqrt, bias=eps_sb
            )
            nc.vector.reciprocal(gmr[:G, 1:2], gmr[:G, 1:2])

            # expand to channels, then per-channel affine A,B:
            # h = A*x + B with A = gamma*rstd, B = beta - mu*A
            AB = persist.tile([P, CT, 2], F32, tag="AB")
            for t in range(CT):
                pexp_t = ps_acc.tile([P, 512], F32, tag="acc", name="pexp")
                pexp = pexp_t[:, 0:2]
                nc.tensor.matmul(
                    pexp, lhsT=gexp_sb[:, t, :], rhs=gmr, start=True, stop=True
                )
                nc.vector.tensor_mul(AB[:, t, 0:1], gnw_sb[:, t, 0:1], pexp[:, 1:2])
                nc.vector.tensor_mul(AB[:, t, 1:2], pexp[:, 0:1], AB[:, t, 0:1])
                nc.vector.tensor_tensor(
                    AB[:, t, 1:2],
                    gnw_sb[:, t, 1:2],
                    AB[:, t, 1:2],
                    op=mybir.AluOpType.subtract,
                )

            # ------------- phase 2+3: normalize + QKV, per 512-col chunk -------------
            for nch in range(NCH):
                hch = work.tile(
                    [P, CT, 512], F8, tag="hch", bufs=t_["hch_bufs"], name="hch"
                )
                for t in range(CT):
                    # normalize on the otherwise-idle Pool engine (SBUF->SBUF);
                    # chunk 0 is on the startup critical path, so spread it
                    # across ACT/DVE/Pool to cut the first-matmul latency
                    if nch == 0 and t == 0:
                        nc.scalar.activation(
                            hch[:, t, :],
                            x16[nch][:, t, :],
                            mybir.ActivationFunctionType.Identity,
                            scale=AB[:, t, 0:1],
                            bias=AB[:, t, 1:2],
                        )
                    elif nch == 0 and t == 1:
                        nc.vector.tensor_scalar(
                            out=hch[:, t, :],
                            in0=x16[nch][:, t, :],
                            scalar1=AB[:, t, 0:1],
                            scalar2=AB[:, t, 1:2],
                            op0=mybir.AluOpType.mult,
                            op1=mybir.AluOpType.add,
                        )
                    else:
                        nc.gpsimd.tensor_scalar(
                            out=hch[:, t, :],
                            in0=x16[nch][:, t, :],
                            scalar1=AB[:, t, 0:1],
                            scalar2=AB[:, t, 1:2],
                            op0=mybir.AluOpType.mult,
                            op1=mybir.AluOpType.add,
                        )
                if has_qkb:
                    # generic path: per-o evac with bias add, split ACT/DVE
                    for bcol, (kk, dst) in enumerate((("q", Q_sb), ("k", K_sb))):
                        for o in range(CT):
                            pq = ps_acc.tile([P, 512], F32, tag="acc", name="pq")
                            for tp in (0, 2):
                                nc.tensor.matmul(
                                    pq,
                                    lhsT=w8[kk][:, tp : tp + 2, ds(o * P, P)],
                                    rhs=hch[:, tp : tp + 2, :],
                                    start=(tp == 0),
                                    stop=(tp == 2),
                                    perf_mode=DR,
                                )
                            if o < 2:
                                nc.scalar.activation(
                                    dst[:, o, ds(nch * 512, 512)],
                                    pq,
                                    mybir.ActivationFunctionType.Identity,
                                    bias=bqk_sb[:, bcol, o : o + 1],
                                )
                            else:
                                nc.vector.tensor_scalar(
                                    out=dst[:, o, ds(nch * 512, 512)],
                                    in0=pq,
                                    scalar1=bqk_sb[:, bcol, o : o + 1],
                                    scalar2=None,
                                    op0=mybir.AluOpType.add,
                                )
                else:
                    # zero q/k bias: pair-merged 1024-wide evacs, ACT/DVE split
                    alt = 0
                    for kk, dst in (("q", Q_sb), ("k", K_sb)):
                        for op in range(2):
                            pq = ps_s.tile([P, 2, 512], F32, tag="s", name="pq")
                            for half in range(2):
                                o = 2 * op + half
                                for tp in (0, 2):
                                    nc.tensor.matmul(
                                        pq[:, half, :],
                                        lhsT=w8[kk][:, tp : tp + 2, ds(o * P, P)],
                                        rhs=hch[:, tp : tp + 2, :],
                                        start=(tp == 0),
                                        stop=(tp == 2),
                                        perf_mode=DR,
                                    )
                            dst_ap = dst[:, 2 * op : 2 * op + 2, ds(nch * 512, 512)]
                            if alt % 2 == 0:
                                nc.scalar.copy(dst_ap, pq)
                            else:
                                nc.vector.tensor_copy(dst_ap, pq)
                            alt += 1
                for j in range(4):
                    m = nch * 4 + j
                    pv = ps_acc.tile([P, 512], F32, tag="acc", name="pv")
                    for tp in (0, 2):
                        nc.tensor.matmul(
                            pv,
                            lhsT=hch[:, tp : tp + 2, ds(j * P, P)],
                            rhs=w8["v"][:, tp : tp + 2, :],
                            start=(tp == 0),
                            stop=(tp == 2),
                            perf_mode=DR,
                        )
                    if j % 2 == 0:
                        nc.scalar.copy(V_sb[:, m, :], pv)
                    else:
                        nc.vector.tensor_copy(V_sb[:, m, :], pv)

            # ------------- phase 4: attention + projection, per query block -------------
            PDP = max(1, t_["pv_depth"])

            def emit_proj(pend):
                a8s, nbp = pend
                for op in range(2):  # output-channel pairs (o = 2*op, 2*op+1)
                    po = ps_s.tile([P, 2, 512], F32, tag="s", name=f"po{op}")
                    for half in range(2):
                        o = 2 * op + half
                        for cp in range(2):
                            nc.tensor.matmul(
                                po[:, half, :],
                                lhsT=w8["p"][:, 2 * cp : 2 * cp + 2, ds(o * P, P)],
                                rhs=a8s[cp],
                                start=(cp == 0),
                                stop=(cp == 1),
                                perf_mode=DR,
                            )
                    ot = work.tile(
                        [P, 2, 512], F16, tag="ot", bufs=t_["ot_bufs"], name="ot"
                    )
                    nc.vector.tensor_tensor(
                        ot, po, x16[nbp][:, 2 * op : 2 * op + 2, :],
                        op=mybir.AluOpType.add,
                    )
                    if with_cb:
                        for half in range(2):
                            o = 2 * op + half
                            nc.vector.tensor_scalar(
                                out=ot[:, half, :],
                                in0=ot[:, half, :],
                                scalar1=cb_sb[:, o : o + 1],
                                scalar2=None,
                                op0=mybir.AluOpType.add,
                            )
                    nc.sync.dma_start(
                        out=out_d.ap()[nbp][:, 2 * op : 2 * op + 2, :], in_=ot
                    )

            # Flat software-pipelined stream over (block, key-pair): the PV
            # matmuls lag the S matmuls by PDP pairs ACROSS block boundaries,
            # so the PE never fronts an exp-dependent PV against the next
            # block's score tiles and ACT streams exps with no boundary gap.
            ST = [None] * NCH  # per-block pipeline state

            def pv1(s, j, last):
                es2 = s["es"][j]
                for c in range(3):
                    nc.tensor.matmul(
                        s["pa"][c],
                        lhsT=V_sb[:, 2 * j : 2 * j + 2, ds(c * P, P)],
                        rhs=es2,
                        start=(j == 0),
                        stop=last,
                        perf_mode=DR,
                    )
                if s["psd"] is None:
                    # lazy: the previous block's pass 2 shares this bank
                    s["psd"] = ps_d.tile([P, 512], F32, tag="d", name="psd")
                nc.tensor.matmul(
                    s["psd"][0:1, :],
                    lhsT=ones8[:, :, 0:1],
                    rhs=es2,
                    start=(j == 0),
                    stop=last,
                    perf_mode=DR,
                )

            def rd_chain(s):
                rdrow = work.tile([1, 512], F32, tag="rdrow", bufs=2, name="rdrow")
                nc.vector.reciprocal(rdrow, s["psd"][0:1, :])
                rd = work.tile([P, 512], F32, tag="rd", bufs=2, name="rd")
                nc.gpsimd.partition_broadcast(rd, rdrow)
                # evac A normalized straight to fp8 pairs for the projection
                a8s = [
                    work.tile(
                        [P, 2, 512], F8, tag="a8", bufs=t_["a8_bufs"],
                        name=f"a8_{cp}",
                    )
                    for cp in range(2)
                ]
                for c in range(3):
                    nc.vector.tensor_mul(a8s[c // 2][:, c % 2, :], s["pa"][c], rd)
                s["rd"], s["a8s"] = rd, a8s

            def pass2_mms(s, pa3_t, js):
                for j in js:
                    nc.tensor.matmul(
                        pa3_t,
                        lhsT=V_sb[:, 2 * j : 2 * j + 2, ds(3 * P, P)],
                        rhs=s["es"][j],
                        start=(j == 0),
                        stop=(j == NP - 1),
                        perf_mode=DR,
                    )

            def pass2(s):
                # PV pass 2 (chunk 3) reuses the D bank, freed by the recip
                pa3_t = ps_d.tile([P, 512], F32, tag="d", name="pa3")
                pass2_mms(s, pa3_t, range(NP))
                nc.vector.tensor_mul(s["a8s"][1][:, 1, :], pa3_t, s["rd"])

            def pipe_step(i):
                # lagging emissions for flat index i (S-mms of i were emitted).
                # pass2 must precede this block's first pv1: its pa3 shares
                # the ps_d bank with this block's lazily-allocated psd.
                nb, j = divmod(i, NP)
                if nb > 0:
                    if j == max(1, PDP):
                        pass2(ST[nb - 1])
                    elif j == max(3, PDP + 2):
                        emit_proj((ST[nb - 1]["a8s"], nb - 1))
                if i >= PDP:
                    onb, oj = divmod(i - PDP, NP)
                    pv1(ST[onb], oj, last=(oj == NP - 1))
                    if oj == NP - 1:
                        rd_chain(ST[onb])

            for i in range(NCH * NP):
                nb, j = divmod(i, NP)
                if j == 0:
                    ST[nb] = dict(
                        pa=[
                            ps_acc.tile([P, 512], F32, tag="acc", name=f"pa{c}")
                            for c in range(3)
                        ],
                        psd=None,
                        es=[None] * NP,
                    )
                s = ST[nb]
                st = ps_s.tile([P, 2, 512], F32, tag="s", name="st")
                for half in range(2):
                    m = 2 * j + half
                    for tp in (0, 2):
                        nc.tensor.matmul(
                            st[:, half, :],
                            lhsT=K_sb[:, tp : tp + 2, ds(m * P, P)],
                            rhs=Q_sb[:, tp : tp + 2, ds(nb * 512, 512)],
                            start=(tp == 0),
                            stop=(tp == 2),
                            perf_mode=DR,
                        )
                pipe_step(i)
                es2 = work.tile(
                    [P, 2, 512], F8, tag="es", bufs=t_["es_bufs"], name="es"
                )
                # one merged 1024-wide exp per key-tile pair
                nc.scalar.activation(
                    es2, st, mybir.ActivationFunctionType.Exp,
                    scale=SCALE, bias=mln16,
                )
                s["es"][j] = es2
            for i in range(NCH * NP, NCH * NP + PDP):
                onb, oj = divmod(i - PDP, NP)
                pv1(ST[onb], oj, last=(oj == NP - 1))
                if oj == NP - 1:
                    rd_chain(ST[onb])
            pass2(ST[NCH - 1])
            emit_proj((ST[NCH - 1]["a8s"], NCH - 1))

    nc.compile()
    return nc


_NC_CACHE = {}


def get_nc(with_cb: bool, tune=None, has_qkb: bool = False):
    key = (with_cb, has_qkb, tuple(sorted((tune or {}).items())))
    if key not in _NC_CACHE:
        _NC_CACHE[key] = build_nc(with_cb, tune, has_qkb)
    return _NC_CACHE[key]


def make_in_maps(x, gn_gamma, gn_beta, wq, bq, wk, bk, wv, bv, wp, bp):
    """Host-side prep: returns (in_maps list for 8 cores, with_cb flag)."""
    x = np.asarray(x, dtype=np.float32)
    B = x.shape[0]
    assert x.shape == (8, C, 64, 64)

    f32 = np.float32
    wqT = np.ascontiguousarray(np.asarray(wq, f32).T).astype(np.float16)
    wkT = np.ascontiguousarray(np.asarray(wk, f32).T).astype(np.float16)
    wvT = np.ascontiguousarray(np.asarray(wv, f32).T).astype(np.float16)
    wpT = np.ascontiguousarray(np.asarray(wp, f32).T).astype(np.float16)

    bq = np.asarray(bq, f32)
    bk = np.asarray(bk, f32)
    bqk = np.ascontiguousarray(
        np.stack([bq.reshape(CT, P).T, bk.reshape(CT, P).T], axis=1)
    )  # [P, 2, CT]
    gnw = np.ascontiguousarray(
        np.stack(
            [np.asarray(gn_gamma, f32).reshape(CT, P).T,
             np.asarray(gn_beta, f32).reshape(CT, P).T],
            axis=2,
        )
    )  # [P, CT, 2]

    gstat = np.zeros((P, CT, G), f32)
    for t in range(CT):
        for p in range(P):
            gstat[p, t, 8 * t + p // GS] = 1.0 / GS
    gexp = np.zeros((P, CT, P), f32)  # [g(padded to 128), t, c]
    for t in range(CT):
        for c in range(P):
            gexp[8 * t + c // GS, t, c] = 1.0

    cb = (np.asarray(wp, f32) @ np.asarray(bv, f32) + np.asarray(bp, f32)).astype(f32)
    with_cb = bool(np.abs(cb).max() > 0)
    has_qkb = bool(np.abs(bqk).max() > 0)
    cb4 = np.ascontiguousarray(cb.reshape(CT, P).T)  # [P, CT]

    shared = {
        "wqT": wqT, "wkT": wkT, "wvT": wvT, "wpT": wpT,
        "bqk": bqk, "gnw": gnw, "gstat": gstat, "gexp": gexp,
    }
    if with_cb:
        shared["cb"] = cb4

    in_maps = []
    for b in range(B):
        m = dict(shared)
        xb = x[b].reshape(C, N).reshape(CT, P, N)
        # [NCH, P, CT, 512]: chunk nb holds columns nb*512..+512, contiguous
        xc = np.ascontiguousarray(
            xb.reshape(CT, P, NCH, 512).transpose(2, 1, 0, 3)
        )
        m["xc16"] = xc.astype(np.float16)
        in_maps.append(m)
    return in_maps, with_cb, has_qkb


def kernel(x, gn_gamma, gn_beta, wq, bq, wk, bk, wv, bv, wp, bp):
    in_maps, with_cb, has_qkb = make_in_maps(
        x, gn_gamma, gn_beta, wq, bq, wk, bk, wv, bv, wp, bp
    )
    nc = get_nc(with_cb, None, has_qkb)
    res = run_bass_kernel_spmd(nc, in_maps, core_ids=list(range(8)))
    outs = []
    for b in range(8):
        oc = res.results[b]["outc"]  # [NCH, P, CT, 512] fp16
        outs.append(np.transpose(oc, (2, 1, 0, 3)).reshape(C, 64, 64))
    return np.stack(outs).astype(np.float32)


# revision 55
# speedup vs baseline: 1.0258x; 1.0258x over previous
"""AttnBlock (GroupNorm + 1x1-conv self-attention + residual) on 8 trn2 cores.

Sharding: pure data parallel -- batch b runs on core b (B=8, n_cores=8).
Each core computes its whole batch: GN -> QKV -> attention -> proj -> residual.

Layout (per core, C=512, N=H*W=4096):
  x resident in SBUF as fp16 [P, CT, 512] x 8 chunks (stats, normalize and
  the residual all read it; x is DMA'd exactly once).
  h, Q, K stored [C, N] (channels on partitions), V stored [N, C]; all fp8e4m3.
  S^T = K^T Q computed [keys m, queries n] in fp8 DoubleRow (K=256/matmul);
  merged score tiles [P, 2, 512] (two key-tiles) so each softmax exp is one
  1024-wide ACT instruction; expS stored as exp(s)/16 in fp8 (cancels in A/D).
  Softmax denominator D via DoubleRow matmuls with a ones lhsT (PE, not DVE);
  division deferred: A_norm = pa * (1/D) fused into the PSUM evacuation
  (DVE), so the projection runs on normalized fp8 A and the residual is a
  single add.
  PV runs in two passes: chunks 0-2 interleaved with the S/exp stream
  (3 PSUM banks), chunk 3 after the stream in the denominator's bank (its
  reciprocal frees it) -- exactly 8 PSUM banks with double-buffered merged
  score tiles.
  Attention is one flat software-pipelined stream over (block, key-pair):
  PV matmuls lag the S matmuls by pv_depth pairs ACROSS block boundaries,
  and the previous block's pass 2 / projection are emitted at fixed
  positions inside the next block's stream, so neither the PE nor ACT
  idles at block boundaries waiting on the reciprocal/broadcast chain.

Engine budget per attention block: ACT 16 merged exps; PE S/PV/D/proj
matmuls (all fp8 DoubleRow); DVE evacuations; Pool broadcasts + normalize.
"""

import sys

sys.path.insert(0, "/opt/trn_rl_repo")

import numpy as np

import concourse.bass as bass
from concourse import bacc
import concourse.tile as tile
from concourse import bass_isa, mybir
from concourse.bass import ds
from concourse.bass_utils import run_bass_kernel_spmd

P = 128
C = 512
CT = C // P          # 4 channel tiles
N = 4096
NCH = N // 512       # 8 column chunks of 512
NT = N // P          # 32 position tiles of 128
NP = NT // 2         # 16 merged key-tile pairs
G = 32               # groups
GS = C // G          # 16 channels per group
EPS = 1e-5
SCALE = float(C) ** -0.5

F16 = mybir.dt.float16
F32 = mybir.dt.float32
F8 = mybir.dt.float8e4
LN16 = 2.772588722239781  # ln(16): expS stored as exp(s)/16 in fp8 (cancels in A/D)

TUNE = dict(
    x16_bufs=8,
    acc_bufs=3,
    s_bufs=2,
    es_bufs=20,
    a8_bufs=6,
    ot_bufs=3,
    hch_bufs=3,
    pv_depth=5,      # S->exp->PV pipeline depth, in merged pairs
    rep=1,           # repeat the whole computation inside the program (timing)
)


def build_nc(with_cb: bool, tune=None, has_qkb: bool = False):
    t_ = dict(TUNE)
    if tune:
        t_.update(tune)
    nc = bacc.Bacc(trn_type="TRN2")

    xc16_d = nc.dram_tensor("xc16", [NCH, P, CT, 512], F16, kind="ExternalInput")
    wq_d = nc.dram_tensor("wqT", [C, C], F16, kind="ExternalInput")
    wk_d = nc.dram_tensor("wkT", [C, C], F16, kind="ExternalInput")
    wv_d = nc.dram_tensor("wvT", [C, C], F16, kind="ExternalInput")
    wp_d = nc.dram_tensor("wpT", [C, C], F16, kind="ExternalInput")
    bqk_d = nc.dram_tensor("bqk", [P, 2, CT], F32, kind="ExternalInput")
    gnw_d = nc.dram_tensor("gnw", [P, CT, 2], F32, kind="ExternalInput")
    gstat_d = nc.dram_tensor("gstat", [P, CT, G], F32, kind="ExternalInput")
    gexp_d = nc.dram_tensor("gexp", [P, CT, P], F32, kind="ExternalInput")
    if with_cb:
        cb_d = nc.dram_tensor("cb", [P, CT], F32, kind="ExternalInput")
    out_d = nc.dram_tensor("outc", [NCH, P, CT, 512], F16, kind="ExternalOutput")
    w3 = {
        "q": wq_d.rearrange("(t p) o -> p t o", p=P),
        "k": wk_d.rearrange("(t p) o -> p t o", p=P),
        "v": wv_d.rearrange("(t p) o -> p t o", p=P),
        "p": wp_d.rearrange("(t p) o -> p t o", p=P),
    }

    from contextlib import ExitStack

    DR = mybir.MatmulPerfMode.DoubleRow

    with tile.TileContext(nc) as tc, ExitStack() as ctx:
        persist = ctx.enter_context(tc.tile_pool(name="persist", bufs=1))
        work = ctx.enter_context(tc.tile_pool(name="work", bufs=2))
        # PSUM: exactly 8 banks.
        #   ps_s:   [P,2,512] x2 = 4 banks (merged score tiles / proj outputs)
        #   ps_acc: [P,512]   x3 = 3 banks (QKV accums; PV pass-1 chunks 0-2)
        #   ps_d:   [P,512]   x1 = 1 bank  (GN stats; softmax D; PV pass-2)
        ps_s = ctx.enter_context(
            tc.tile_pool(name="ps_s", bufs=t_["s_bufs"], space="PSUM")
        )
        ps_acc = ctx.enter_context(
            tc.tile_pool(name="ps_acc", bufs=t_["acc_bufs"], space="PSUM")
        )
        ps_d = ctx.enter_context(tc.tile_pool(name="ps_d", bufs=1, space="PSUM"))

        # DMA issue cost serializes on the ISSUING engine's queue, so spread
        # the startup transfers: x chunks on SP, weights on Pool (which also
        # converts them to fp8), GN constants on ACT (idle before its stats
        # work). Everything gates on stats, which gate on the x stream.
        # x chunks as half-chunk pair tiles [P,2,512]: dependency granularity
        # lets the DVE stats pass start on tiles 0-1 of a chunk while 2-3 are
        # still in flight, and the pairs match the residual-add reads.
        def dma_x16():
            xs = []
            for nch in range(NCH):
                xa = work.tile(
                    [P, 2, 512], F16, tag="x16", bufs=2 * t_["x16_bufs"],
                    name="x16a",
                )
                nc.sync.dma_start(out=xa, in_=xc16_d.ap()[nch][:, 0:2, :])
                xb = work.tile(
                    [P, 2, 512], F16, tag="x16", bufs=2 * t_["x16_bufs"],
                    name="x16b",
                )
                nc.sync.dma_start(out=xb, in_=xc16_d.ap()[nch][:, 2:4, :])
                xs.append((xa, xb))
            return xs

        def x16t(nch, t):
            return x16[nch][t // 2][:, t % 2, :]

        x16 = dma_x16()

        # ---------------- persistent weights / constants ----------------
        wsb = {}
        for kk in ("q", "k", "v", "p"):
            wsb[kk] = work.tile([P, CT, C], F16, tag="w16", bufs=4, name=f"w16{kk}")
            nc.gpsimd.dma_start(out=wsb[kk], in_=w3[kk])
        w8 = {}
        for kk in ("q", "k", "v", "p"):
            # fp8 weight copies, converted on the idle Pool engine at startup;
            # consumed by DoubleRow matmuls as channel-tile pairs.
            w8[kk] = persist.tile([P, CT, C], F8, tag=f"w8{kk}", name=f"w8{kk}")
            nc.gpsimd.tensor_copy(w8[kk], wsb[kk])
        bqk_sb = persist.tile([P, 2, CT], F32, tag="bqk")
        nc.scalar.dma_start(out=bqk_sb, in_=bqk_d.ap())
        gnw_sb = persist.tile([P, CT, 2], F32, tag="gnw")
        nc.scalar.dma_start(out=gnw_sb, in_=gnw_d.ap())
        gstat_sb = persist.tile([P, CT, G], F32, tag="gstat")
        nc.scalar.dma_start(out=gstat_sb, in_=gstat_d.ap())
        gexp_sb = persist.tile([P, CT, P], F32, tag="gexp")
        nc.scalar.dma_start(out=gexp_sb, in_=gexp_d.ap())
        if with_cb:
            cb_sb = persist.tile([P, CT], F32, tag="cb")
            nc.scalar.dma_start(out=cb_sb, in_=cb_d.ap())
        eps_sb = persist.tile([G, 1], F32, tag="eps")
        nc.vector.memset(eps_sb, EPS)
        ones8 = persist.tile([P, 2, 16], F8, tag="ones8")
        nc.vector.memset(ones8, 1.0)
        mln16 = persist.tile([P, 1], F32, tag="mln16")
        nc.vector.memset(mln16, -LN16)

        Q_sb = persist.tile([P, CT, N], F8, tag="Q")
        K_sb = persist.tile([P, CT, N], F8, tag="K")
        V_sb = persist.tile([P, NT, C], F8, tag="V")

        SD = nc.vector.BN_STATS_DIM
        AD = nc.vector.BN_AGGR_DIM

        for _rep in range(t_["rep"]):
            # ---------------- phase 1: GN statistics ----------------
            stats = persist.tile([P, CT - 1, NCH, SD], F32, tag="stats")
            if _rep > 0:
                x16 = dma_x16()
            # tiles 0..2 on DVE (bn_stats); tile 3 on the otherwise-idle ACT
            # via accum_out (sum + sum of squares).
            asum = persist.tile([P, NCH, 2], F32, tag="asum")
            # DVE pass 1: tiles 0,1 (gated on each chunk's first half only);
            # DVE pass 2: tile 2; ACT: tile 3 via accum_out (sum + sumsq)
            for nch in range(NCH):
                for t in range(2):
                    nc.vector.bn_stats(
                        out=stats[:, t, nch, :], in_=x16t(nch, t)
                    )
                ascr = work.tile([P, 512], F16, tag="ascr", bufs=2, name="ascr")
                nc.scalar.activation(
                    ascr,
                    x16t(nch, CT - 1),
                    mybir.ActivationFunctionType.Copy,
                    accum_out=asum[:, nch, 0:1],
                )
                ascr2 = work.tile([P, 512], F16, tag="ascr", bufs=2, name="ascr2")
                nc.scalar.activation(
                    ascr2,
                    x16t(nch, CT - 1),
                    mybir.ActivationFunctionType.Square,
                    accum_out=asum[:, nch, 1:2],
                )
            for nch in range(NCH):
                nc.vector.bn_stats(out=stats[:, 2, nch, :], in_=x16t(nch, 2))
            mv = persist.tile([P, CT, AD], F32, tag="mv")
            rs = persist.tile([P, CT, 2], F32, tag="rs")
            for t in range(CT - 1):
                nc.vector.bn_aggr(out=mv[:, t, :], in_=stats[:, t, :, :])
                # rs = [mean, E[x^2]] per channel
                nc.vector.tensor_mul(rs[:, t, 1:2], mv[:, t, 0:1], mv[:, t, 0:1])
                nc.vector.tensor_add(rs[:, t, 1:2], rs[:, t, 1:2], mv[:, t, 1:2])
                nc.vector.tensor_copy(rs[:, t, 0:1], mv[:, t, 0:1])
            # tile 3: rs = [sum, sumsq]/N from the ACT accumulators
            ssum = persist.tile([P, 2], F32, tag="ssum")
            nc.vector.reduce_sum(
                out=ssum[:, 0:1], in_=asum[:, :, 0], axis=mybir.AxisListType.X
            )
            nc.vector.reduce_sum(
                out=ssum[:, 1:2], in_=asum[:, :, 1], axis=mybir.AxisListType.X
            )
            nc.vector.tensor_scalar_mul(rs[:, CT - 1, :], ssum, 1.0 / N)

            # group reduce: pstat[g, :] = [mu_g, E2_g]  (1/GS folded into gstat)
            pstat_t = ps_d.tile([P, 512], F32, tag="d", name="pstat")
            pstat = pstat_t[:G, 0:2]
            for t in range(CT):
                nc.tensor.matmul(
                    pstat,
                    lhsT=gstat_sb[:, t, :],
                    rhs=rs[:, t, :],
                    start=(t == 0),
                    stop=(t == CT - 1),
                )
            gmr = persist.tile([P, 2], F32, tag="gmr")  # [mu_g, rstd_g], padded
            nc.vector.memset(gmr, 0.0)
            gst = persist.tile([G, 2], F32, tag="gst")  # pstat evacuated to SBUF
            nc.vector.tensor_copy(gst, pstat)
            gvar = persist.tile([G, 1], F32, tag="gvar")
            nc.vector.tensor_copy(gmr[:G, 0:1], gst[:, 0:1])
            nc.vector.tensor_mul(gvar, gst[:, 0:1], gst[:, 0:1])
            nc.vector.tensor_tensor(
                gvar, gst[:, 1:2], gvar, op=mybir.AluOpType.subtract
            )
            nc.scalar.activation(
                gmr[:G, 1:2], gvar, mybir.ActivationFunctionType.Sqrt, bias=eps_sb
            )
            nc.vector.reciprocal(gmr[:G, 1:2], gmr[:G, 1:2])

            # expand to channels, then per-channel affine A,B:
            # h = A*x + B with A = gamma*rstd, B = beta - mu*A
            AB = persist.tile([P, CT, 2], F32, tag="AB")
            for t in range(CT):
                pexp_t = ps_acc.tile([P, 512], F32, tag="acc", name="pexp")
                pexp = pexp_t[:, 0:2]
                nc.tensor.matmul(
                    pexp, lhsT=gexp_sb[:, t, :], rhs=gmr, start=True, stop=True
                )
                nc.vector.tensor_mul(AB[:, t, 0:1], gnw_sb[:, t, 0:1], pexp[:, 1:2])
                nc.vector.tensor_mul(AB[:, t, 1:2], pexp[:, 0:1], AB[:, t, 0:1])
                nc.vector.tensor_tensor(
                    AB[:, t, 1:2],
                    gnw_sb[:, t, 1:2],
                    AB[:, t, 1:2],
                    op=mybir.AluOpType.subtract,
                )

            # ------------- phase 2+3: normalize + QKV, per 512-col chunk -------------
            for nch in range(NCH):
                hch = work.tile(
                    [P, CT, 512], F8, tag="hch", bufs=t_["hch_bufs"], name="hch"
                )
                for t in range(CT):
                    # normalize on the otherwise-idle Pool engine (SBUF->SBUF);
                    # chunk 0 is on the startup critical path, so spread it
                    # across ACT/DVE/Pool to cut the first-matmul latency
                    if nch == 0 and t == 0:
                        nc.scalar.activation(
                            hch[:, t, :],
                            x16t(nch, t),
                            mybir.ActivationFunctionType.Identity,
                            scale=AB[:, t, 0:1],
                            bias=AB[:, t, 1:2],
                        )
                    elif nch == 0 and t == 1:
                        nc.vector.tensor_scalar(
                            out=hch[:, t, :],
                            in0=x16t(nch, t),
                            scalar1=AB[:, t, 0:1],
                            scalar2=AB[:, t, 1:2],
                            op0=mybir.AluOpType.mult,
                            op1=mybir.AluOpType.add,
                        )
                    else:
                        nc.gpsimd.tensor_scalar(
                            out=hch[:, t, :],
                            in0=x16t(nch, t),
                            scalar1=AB[:, t, 0:1],
                            scalar2=AB[:, t, 1:2],
                            op0=mybir.AluOpType.mult,
                            op1=mybir.AluOpType.add,
                        )
                if has_qkb:
                    # generic path: per-o evac with bias add, split ACT/DVE
                    for bcol, (kk, dst) in enumerate((("q", Q_sb), ("k", K_sb))):
                        for o in range(CT):
                            pq = ps_acc.tile([P, 512], F32, tag="acc", name="pq")
                            for tp in (0, 2):
                                nc.tensor.matmul(
                                    pq,
                                    lhsT=w8[kk][:, tp : tp + 2, ds(o * P, P)],
                                    rhs=hch[:, tp : tp + 2, :],
                                    start=(tp == 0),
                                    stop=(tp == 2),
                                    perf_mode=DR,
                                )
                            if o < 2:
                                nc.scalar.activation(
                                    dst[:, o, ds(nch * 512, 512)],
                                    pq,
                                    mybir.ActivationFunctionType.Identity,
                                    bias=bqk_sb[:, bcol, o : o + 1],
                                )
                            else:
                                nc.vector.tensor_scalar(
                                    out=dst[:, o, ds(nch * 512, 512)],
                                    in0=pq,
                                    scalar1=bqk_sb[:, bcol, o : o + 1],
                                    scalar2=None,
                                    op0=mybir.AluOpType.add,
                                )
                else:
                    # zero q/k bias: pair-merged 1024-wide evacs, ACT/DVE split
                    alt = 0
                    for kk, dst in (("q", Q_sb), ("k", K_sb)):
                        for op in range(2):
                            pq = ps_s.tile([P, 2, 512], F32, tag="s", name="pq")
                            for half in range(2):
                                o = 2 * op + half
                                for tp in (0, 2):
                                    nc.tensor.matmul(
                                        pq[:, half, :],
                                        lhsT=w8[kk][:, tp : tp + 2, ds(o * P, P)],
                                        rhs=hch[:, tp : tp + 2, :],
                                        start=(tp == 0),
                                        stop=(tp == 2),
                                        perf_mode=DR,
                                    )
                            dst_ap = dst[:, 2 * op : 2 * op + 2, ds(nch * 512, 512)]
                            if alt % 2 == 0:
                                nc.scalar.copy(dst_ap, pq)
                            else:
                                nc.vector.tensor_copy(dst_ap, pq)
                            alt += 1
                for j in range(4):
                    m = nch * 4 + j
                    pv = ps_acc.tile([P, 512], F32, tag="acc", name="pv")
                    for tp in (0, 2):
                        nc.tensor.matmul(
                            pv,
                            lhsT=hch[:, tp : tp + 2, ds(j * P, P)],
                            rhs=w8["v"][:, tp : tp + 2, :],
                            start=(tp == 0),
                            stop=(tp == 2),
                            perf_mode=DR,
                        )
                    if j % 2 == 0:
                        nc.scalar.copy(V_sb[:, m, :], pv)
                    else:
                        nc.vector.tensor_copy(V_sb[:, m, :], pv)

            # ------------- phase 4: attention + projection, per query block -------------
            PDP = max(1, t_["pv_depth"])

            def emit_proj(pend):
                a8s, nbp = pend
                for op in range(2):  # output-channel pairs (o = 2*op, 2*op+1)
                    po = ps_s.tile([P, 2, 512], F32, tag="s", name=f"po{op}")
                    for half in range(2):
                        o = 2 * op + half
                        for cp in range(2):
                            nc.tensor.matmul(
                                po[:, half, :],
                                lhsT=w8["p"][:, 2 * cp : 2 * cp + 2, ds(o * P, P)],
                                rhs=a8s[cp],
                                start=(cp == 0),
                                stop=(cp == 1),
                                perf_mode=DR,
                            )
                    ot = work.tile(
                        [P, 2, 512], F16, tag="ot", bufs=t_["ot_bufs"], name="ot"
                    )
                    nc.vector.tensor_tensor(
                        ot, po, x16[nbp][op], op=mybir.AluOpType.add,
                    )
                    if with_cb:
                        for half in range(2):
                            o = 2 * op + half
                            nc.vector.tensor_scalar(
                                out=ot[:, half, :],
                                in0=ot[:, half, :],
                                scalar1=cb_sb[:, o : o + 1],
                                scalar2=None,
                                op0=mybir.AluOpType.add,
                            )
                    nc.sync.dma_start(
                        out=out_d.ap()[nbp][:, 2 * op : 2 * op + 2, :], in_=ot
                    )

            # Flat software-pipelined stream over (block, key-pair): the PV
            # matmuls lag the S matmuls by PDP pairs ACROSS block boundaries,
            # so the PE never fronts an exp-dependent PV against the next
            # block's score tiles and ACT streams exps with no boundary gap.
            ST = [None] * NCH  # per-block pipeline state

            def pv1(s, j, last):
                es2 = s["es"][j]
                for c in range(3):
                    nc.tensor.matmul(
                        s["pa"][c],
                        lhsT=V_sb[:, 2 * j : 2 * j + 2, ds(c * P, P)],
                        rhs=es2,
                        start=(j == 0),
                        stop=last,
                        perf_mode=DR,
                    )
                if s["psd"] is None:
                    # lazy: the previous block's pass 2 shares this bank
                    s["psd"] = ps_d.tile([P, 512], F32, tag="d", name="psd")
                nc.tensor.matmul(
                    s["psd"][0:1, :],
                    lhsT=ones8[:, :, 0:1],
                    rhs=es2,
                    start=(j == 0),
                    stop=last,
                    perf_mode=DR,
                )

            def rd_chain(s):
                rdrow = work.tile([1, 512], F32, tag="rdrow", bufs=2, name="rdrow")
                nc.vector.reciprocal(rdrow, s["psd"][0:1, :])
                rd = work.tile([P, 512], F32, tag="rd", bufs=2, name="rd")
                nc.gpsimd.partition_broadcast(rd, rdrow)
                # evac A normalized straight to fp8 pairs for the projection
                a8s = [
                    work.tile(
                        [P, 2, 512], F8, tag="a8", bufs=t_["a8_bufs"],
                        name=f"a8_{cp}",
                    )
                    for cp in range(2)
                ]
                for c in range(3):
                    nc.vector.tensor_mul(a8s[c // 2][:, c % 2, :], s["pa"][c], rd)
                s["rd"], s["a8s"] = rd, a8s

            def pass2_mms(s, pa3_t, js):
                for j in js:
                    nc.tensor.matmul(
                        pa3_t,
                        lhsT=V_sb[:, 2 * j : 2 * j + 2, ds(3 * P, P)],
                        rhs=s["es"][j],
                        start=(j == 0),
                        stop=(j == NP - 1),
                        perf_mode=DR,
                    )

            def pass2(s):
                # PV pass 2 (chunk 3) reuses the D bank, freed by the recip
                pa3_t = ps_d.tile([P, 512], F32, tag="d", name="pa3")
                pass2_mms(s, pa3_t, range(NP))
                nc.vector.tensor_mul(s["a8s"][1][:, 1, :], pa3_t, s["rd"])

            def pipe_step(i):
                # lagging emissions for flat index i (S-mms of i were emitted).
                # pass2 must precede this block's first pv1: its pa3 shares
                # the ps_d bank with this block's lazily-allocated psd.
                nb, j = divmod(i, NP)
                if nb > 0:
                    if j == max(1, PDP):
                        pass2(ST[nb - 1])
                    elif j == max(3, PDP + 2):
                        emit_proj((ST[nb - 1]["a8s"], nb - 1))
                if i >= PDP:
                    onb, oj = divmod(i - PDP, NP)
                    pv1(ST[onb], oj, last=(oj == NP - 1))
                    if oj == NP - 1:
                        rd_chain(ST[onb])

            for i in range(NCH * NP):
                nb, j = divmod(i, NP)
                if j == 0:
                    ST[nb] = dict(
                        pa=[
                            ps_acc.tile([P, 512], F32, tag="acc", name=f"pa{c}")
                            for c in range(3)
                        ],
                        psd=None,
                        es=[None] * NP,
                    )
                s = ST[nb]
                st = ps_s.tile([P, 2, 512], F32, tag="s", name="st")
                for half in range(2):
                    m = 2 * j + half
                    for tp in (0, 2):
                        nc.tensor.matmul(
                            st[:, half, :],
                            lhsT=K_sb[:, tp : tp + 2, ds(m * P, P)],
                            rhs=Q_sb[:, tp : tp + 2, ds(nb * 512, 512)],
                            start=(tp == 0),
                            stop=(tp == 2),
                            perf_mode=DR,
                        )
                es2 = work.tile(
                    [P, 2, 512], F8, tag="es", bufs=t_["es_bufs"], name="es"
                )
                # one merged 1024-wide exp per key-tile pair; emitted BEFORE
                # the lagging pv/pass2/proj so its PE-counter wait covers
                # only its own score matmuls
                nc.scalar.activation(
                    es2, st, mybir.ActivationFunctionType.Exp,
                    scale=SCALE, bias=mln16,
                )
                s["es"][j] = es2
                pipe_step(i)
            for i in range(NCH * NP, NCH * NP + PDP):
                onb, oj = divmod(i - PDP, NP)
                pv1(ST[onb], oj, last=(oj == NP - 1))
                if oj == NP - 1:
                    rd_chain(ST[onb])
            pass2(ST[NCH - 1])
            emit_proj((ST[NCH - 1]["a8s"], NCH - 1))

    nc.compile()
    return nc


_NC_CACHE = {}


def get_nc(with_cb: bool, tune=None, has_qkb: bool = False):
    key = (with_cb, has_qkb, tuple(sorted((tune or {}).items())))
    if key not in _NC_CACHE:
        _NC_CACHE[key] = build_nc(with_cb, tune, has_qkb)
    return _NC_CACHE[key]


def make_in_maps(x, gn_gamma, gn_beta, wq, bq, wk, bk, wv, bv, wp, bp):
    """Host-side prep: returns (in_maps list for 8 cores, with_cb flag)."""
    x = np.asarray(x, dtype=np.float32)
    B = x.shape[0]
    assert x.shape == (8, C, 64, 64)

    f32 = np.float32
    wqT = np.ascontiguousarray(np.asarray(wq, f32).T).astype(np.float16)
    wkT = np.ascontiguousarray(np.asarray(wk, f32).T).astype(np.float16)
    wvT = np.ascontiguousarray(np.asarray(wv, f32).T).astype(np.float16)
    wpT = np.ascontiguousarray(np.asarray(wp, f32).T).astype(np.float16)

    bq = np.asarray(bq, f32)
    bk = np.asarray(bk, f32)
    bqk = np.ascontiguousarray(
        np.stack([bq.reshape(CT, P).T, bk.reshape(CT, P).T], axis=1)
    )  # [P, 2, CT]
    gnw = np.ascontiguousarray(
        np.stack(
            [np.asarray(gn_gamma, f32).reshape(CT, P).T,
             np.asarray(gn_beta, f32).reshape(CT, P).T],
            axis=2,
        )
    )  # [P, CT, 2]

    gstat = np.zeros((P, CT, G), f32)
    for t in range(CT):
        for p in range(P):
            gstat[p, t, 8 * t + p // GS] = 1.0 / GS
    gexp = np.zeros((P, CT, P), f32)  # [g(padded to 128), t, c]
    for t in range(CT):
        for c in range(P):
            gexp[8 * t + c // GS, t, c] = 1.0

    cb = (np.asarray(wp, f32) @ np.asarray(bv, f32) + np.asarray(bp, f32)).astype(f32)
    with_cb = bool(np.abs(cb).max() > 0)
    has_qkb = bool(np.abs(bqk).max() > 0)
    cb4 = np.ascontiguousarray(cb.reshape(CT, P).T)  # [P, CT]

    shared = {
        "wqT": wqT, "wkT": wkT, "wvT": wvT, "wpT": wpT,
        "bqk": bqk, "gnw": gnw, "gstat": gstat, "gexp": gexp,
    }
    if with_cb:
        shared["cb"] = cb4

    in_maps = []
    for b in range(B):
        m = dict(shared)
        xb = x[b].reshape(C, N).reshape(CT, P, N)
        # [NCH, P, CT, 512]: chunk nb holds columns nb*512..+512, contiguous
        xc = np.ascontiguousarray(
            xb.reshape(CT, P, NCH, 512).transpose(2, 1, 0, 3)
        )
        m["xc16"] = xc.astype(np.float16)
        in_maps.append(m)
    return in_maps, with_cb, has_qkb


def kernel(x, gn_gamma, gn_beta, wq, bq, wk, bk, wv, bv, wp, bp):
    in_maps, with_cb, has_qkb = make_in_maps(
        x, gn_gamma, gn_beta, wq, bq, wk, bk, wv, bv, wp, bp
    )
    nc = get_nc(with_cb, None, has_qkb)
    res = run_bass_kernel_spmd(nc, in_maps, core_ids=list(range(8)))
    outs = []
    for b in range(8):
        oc = res.results[b]["outc"]  # [NCH, P, CT, 512] fp16
        outs.append(np.transpose(oc, (2, 1, 0, 3)).reshape(C, 64, 64))
    return np.stack(outs).astype(np.float32)
